# revision 10
# baseline (speedup 1.0000x reference)
"""PolyNet kernel for 8 trn2 NeuronCores (data-parallel over batch).

Algebraic structure exploited (all folds done host-side in float64):
  - The g-branch (og1, xg2, xg3) and the layer-3 f output (xf3) only reach
    the final output through the width-1 projection Wfc.  Folding Wfc into
    those weights collapses their [16k, 128] GEMMs into length-16k vectors,
    whose outer-product parts are 128x128 bilinear forms:
        sum_ij o1_i ot_j G[i,j] = sum_j (o1 @ G)_j * ot_j .
  - Only xf2 = xx2 @ Wf2 survives as a real tensor op (its output feeds
    layer 3's outer product).  Its outer-product part is a bank of 128
    bilinear forms xf2_m = o1^T G_m o1; each symmetrized G_m is
    eigendecomposed on the host:  xf2_m = sum_r s_{m,r} (u_{m,r} . o1)^2.
    On device that is: project z = Vp^T o1 (TensorE), square elementwise
    (single-source op, split across ScalarE/VectorE), then a +-1
    block-diagonal matmul back into PSUM.  No broadcast, no tensor-tensor
    multiply, no PSUM->SBUF copies of wide data.

Device layout is feature-on-partition ("transposed"): activations are
[features, batch_free] so the TensorEngine can contract over features.
"""

import sys
import numpy as np

for _p in ("/opt/trn_rl_repo",):
    if _p not in sys.path:
        sys.path.insert(0, _p)

N_CORES = 8
B, D_IN, NF = 8192, 64, 128
R = B // N_CORES          # rows per core
HB = 512                  # half-batch: free dim of one PSUM-bank unit
NH = R // HB              # 2 halves
NCH = 65                  # projection chunks: 8128 pair-sums + 128 diagonals, padded
NPAD = NCH * NF           # 8320 projections incl. 64 zero-pad
LOOK = 2                  # chunk software-pipeline lookahead (in chunks)
ACT_SHARE = 5             # of every 9 squares, this many go to ScalarE


def _square_op():
    """Custom single-stream DVE square: out = in0^2 with ONE tensor read,
    so VectorE can consume f32 PSUM directly (TensorTensor would need two
    PSUM reads, which the ISA forbids)."""
    from concourse import dve_ops
    from concourse.dve_spec import Spec, Src0, sq

    for op in dve_ops.OPS:
        if op.name == "SQUARE_ANT":
            return op
    op = dve_ops.DveOp(
        "SQUARE_ANT",
        Spec(body=sq(Src0),
             reference=lambda in0, in1, s0, s1, imm2: (in0.astype(np.float32) ** 2)),
        subdim=False,
        uops_sha={"v3": "7bd23a2deee7f188", "v4": "80f1201cc018d83b"},
    )
    dve_ops.OPS.append(op)
    dve_ops._SUB_OPCODE_FOR_NAME[op.name] = (
        dve_ops._CUSTOM_DVE_ROW_BASE + len(dve_ops.OPS) - 1
    )
    return op


def _build_bass():
    import concourse.bacc as bacc
    import concourse.mybir as mybir
    import concourse.tile as tile
    from contextlib import ExitStack

    square = _square_op()

    bf16 = mybir.dt.bfloat16
    f32 = mybir.dt.float32
    AF = mybir.ActivationFunctionType

    nc = bacc.Bacc(
        "TRN2",
        target_bir_lowering=False,
        debug=False,
        enable_asserts=True,
        num_devices=N_CORES,
    )

    def din(name, shape, dt=bf16):
        return nc.dram_tensor(name, shape, dt, kind="ExternalInput").ap()

    xT = din("xT", [D_IN, R])
    w1f = din("w1f", [D_IN, NF])
    b1f = din("b1f", [NF, 1], f32)
    a2f = din("a2f", [NF, NF])
    b2f = din("b2f", [NF, 1], f32)
    vp = din("vp", [NF, NPAD])        # 0/1 pair-indicator projection columns
    s2 = din("s2", [NF, NPAD])        # chunk-major packed polarization coefficients
    g2g = din("g2g", [NF, NF])
    g3f = din("g3f", [NF, NF])
    g3g = din("g3g", [NF, NF])
    ulin = din("ulin", [D_IN, 1])
    vot1 = din("vot1", [NF, 1])
    vxf2 = din("vxf2", [NF, 1])
    out = nc.dram_tensor("out", [1, R], f32, kind="ExternalOutput").ap()

    NGRP = 13
    GRP = NCH // NGRP     # 5 chunks per DMA group

    with tile.TileContext(nc) as tc, ExitStack() as ctx:
        consts = ctx.enter_context(tc.tile_pool(name="consts", bufs=1))
        sb1 = ctx.enter_context(tc.tile_pool(name="sb1", bufs=1))
        zsqp = ctx.enter_context(tc.tile_pool(name="zsq", bufs=LOOK + 4))
        ps_z = ctx.enter_context(tc.tile_pool(name="ps_z", bufs=2, space="PSUM"))
        ps_xf2 = ctx.enter_context(tc.tile_pool(name="ps_xf2", bufs=2, space="PSUM"))
        ps_acc = ctx.enter_context(tc.tile_pool(name="ps_acc", bufs=1, space="PSUM"))

        # ---- constants / weights ----
        xT_sb = consts.tile([D_IN, R], bf16)
        nc.sync.dma_start(xT_sb[:], xT)
        w1f_sb = consts.tile([D_IN, NF], bf16)
        nc.sync.dma_start(w1f_sb[:], w1f)
        b1f_sb = consts.tile([NF, 1], f32)
        nc.sync.dma_start(b1f_sb[:], b1f)
        a2f_sb = consts.tile([NF, NF], bf16)
        nc.sync.dma_start(a2f_sb[:], a2f)
        b2f_sb = consts.tile([NF, 1], f32)
        nc.sync.dma_start(b2f_sb[:], b2f)
        g2g_sb = consts.tile([NF, NF], bf16)
        nc.sync.dma_start(g2g_sb[:], g2g)
        g3f_sb = consts.tile([NF, NF], bf16)
        nc.sync.dma_start(g3f_sb[:], g3f)
        g3g_sb = consts.tile([NF, NF], bf16)
        nc.sync.dma_start(g3g_sb[:], g3g)
        ulin_sb = consts.tile([D_IN, 1], bf16)
        nc.sync.dma_start(ulin_sb[:], ulin)
        vot1_sb = consts.tile([NF, 1], bf16)
        nc.sync.dma_start(vot1_sb[:], vot1)
        vxf2_sb = consts.tile([NF, 1], bf16)
        nc.sync.dma_start(vxf2_sb[:], vxf2)
        ones_sb = consts.tile([NF, 1], bf16)
        nc.vector.memset(ones_sb[:], 1.0)

        vp_g, s2_g = [], []
        for g in range(NGRP):
            vg = consts.tile([NF, GRP * NF], bf16, tag=f"vp{g}")
            nc.gpsimd.dma_start(vg[:], vp[:, g * GRP * NF:(g + 1) * GRP * NF])
            vp_g.append(vg)
            sg = consts.tile([NF, GRP * NF], bf16, tag=f"s2{g}")
            nc.gpsimd.dma_start(sg[:], s2[:, g * GRP * NF:(g + 1) * GRP * NF])
            s2_g.append(sg)

        def chunk_ap(tiles, c):
            g, rr = divmod(c, GRP)
            return tiles[g][:, rr * NF:(rr + 1) * NF]

        # ---- layer 1 + bilinear h tiles (all [NF, R] bf16 in SBUF) ----
        o1 = sb1.tile([NF, R], bf16)
        xf2 = sb1.tile([NF, R], bf16)
        h2g = sb1.tile([NF, R], bf16)
        h3f = sb1.tile([NF, R], bf16)
        h3g = sb1.tile([NF, R], bf16)
        p2g = sb1.tile([NF, R], bf16)
        p3f = sb1.tile([NF, R], bf16)
        p3g = sb1.tile([NF, R], bf16)
        acc_sb = sb1.tile([1, R], f32)

        ps = ps_z.tile([NF, R], f32, tag="z", name="zps")
        for h in range(NH):
            s = slice(h * HB, (h + 1) * HB)
            nc.tensor.matmul(ps[:, s], w1f_sb[:], xT_sb[:, s], start=True, stop=True)
        nc.scalar.activation(o1[:], ps[:], AF.Identity,
                             bias=b1f_sb[:, 0:1], scale=1.0)
        for gw, ht in ((g2g_sb, h2g), (g3f_sb, h3f), (g3g_sb, h3g)):
            ps = ps_z.tile([NF, R], f32, tag="z", name="zps")
            for h in range(NH):
                s = slice(h * HB, (h + 1) * HB)
                nc.tensor.matmul(ps[:, s], gw[:], o1[:, s], start=True, stop=True)
            nc.scalar.copy(ht[:], ps[:])

        nc.vector.tensor_mul(p2g[:], o1[:], h2g[:])

        # ---- output accumulators (one per half) ----
        acc2 = ps_acc.tile([33, HB], f32, tag="acc", name="acc2")
        acc = [acc2[32 * h:32 * h + 1, :] for h in range(NH)]
        for h in range(NH):
            s = slice(h * HB, (h + 1) * HB)
            nc.tensor.matmul(acc[h][:], ulin_sb[:], xT_sb[:, s], start=True, stop=False)
            nc.tensor.matmul(acc[h][:], vot1_sb[:], o1[:, s], start=False, stop=False)
            nc.tensor.matmul(acc[h][:], ones_sb[:], p2g[:, s], start=False, stop=False)

        # ---- xf2 = ot1 @ A2f + sum_r s (u . o1)^2 + b2f ----
        xf2_ps = [ps_xf2.tile([NF, HB], f32, tag="xf2", name=f"xf2ps{h}") for h in range(NH)]
        for h in range(NH):
            s = slice(h * HB, (h + 1) * HB)
            nc.tensor.matmul(xf2_ps[h][:], a2f_sb[:], o1[:, s], start=True, stop=False)

        z_ps = {}
        zsq_sb = {}

        def emit_z(c):
            ps = ps_z.tile([NF, R], f32, tag="z", name="zps")
            for h in range(NH):
                s = slice(h * HB, (h + 1) * HB)
                nc.tensor.matmul(ps[:, s], chunk_ap(vp_g, c), o1[:, s],
                                 start=True, stop=True)
            z_ps[c] = ps

        def emit_square(c):
            ps = z_ps.pop(c)
            zq = zsqp.tile([NF, R], bf16, tag="zsq", name="zsq")
            if c % 2 == 0:
                nc.scalar.square(zq[:], ps[:])
            else:
                nc.vector._custom_dve(square, out=zq[:], in0=ps[:])
            zsq_sb[c] = zq

        def emit_final(c):
            zq = zsq_sb.pop(c)
            for h in range(NH):
                s = slice(h * HB, (h + 1) * HB)
                nc.tensor.matmul(xf2_ps[h][:], chunk_ap(s2_g, c), zq[:, s],
                                 start=False, stop=(c == NCH - 1))

        for c in range(LOOK):
            emit_z(c)
            emit_square(c)
        for c in range(NCH):
            if c + LOOK < NCH:
                emit_z(c + LOOK)
                emit_square(c + LOOK)
            emit_final(c)

        for h in range(NH):
            s = slice(h * HB, (h + 1) * HB)
            nc.scalar.activation(xf2[:, s], xf2_ps[h][:], AF.Identity,
                                 bias=b2f_sb[:, 0:1], scale=1.0)

        # ---- late scalar contributions ----
        nc.vector.tensor_mul(p3f[:], xf2[:], h3f[:])
        nc.vector.tensor_mul(p3g[:], xf2[:], h3g[:])
        for h in range(NH):
            s = slice(h * HB, (h + 1) * HB)
            nc.tensor.matmul(acc[h][:], vxf2_sb[:], xf2[:, s], start=False, stop=False)
            nc.tensor.matmul(acc[h][:], ones_sb[:], p3f[:, s], start=False, stop=False)
            nc.tensor.matmul(acc[h][:], ones_sb[:], p3g[:, s], start=False, stop=True)
            nc.scalar.copy(acc_sb[:, s], acc[h][:])  # [1,HB] from partition 32h
        nc.sync.dma_start(out, acc_sb[:])

    nc.compile()
    return nc


_CACHE = {}


def _get_nc():
    if "nc" not in _CACHE:
        _CACHE["nc"] = _build_bass()
    return _CACHE["nc"]


def _host_fold(inputs):
    import ml_dtypes

    I = {k: np.asarray(v, np.float64) for k, v in inputs.items()}
    x = I["x"]
    bias0 = I["bias0"]
    Wf1, bf1, Wg1, bg1 = I["Wf1"], I["bf1"], I["Wg1"], I["bg1"]
    Wf2, bf2, Wg2, bg2 = I["Wf2"], I["bf2"], I["Wg2"], I["bg2"]
    Wf3, bf3, Wg3, bg3 = I["Wf3"], I["bf3"], I["Wg3"], I["bg3"]
    Wfc, bfc = I["Wfc"], I["bfc"]

    wfc = Wfc[:, 0]
    v_ot1 = wfc[1:129]
    v_f2 = wfc[129:257]
    v_f3 = wfc[257:385]
    v_g1 = wfc[385:513]
    v_g2 = wfc[513:641]
    v_g3 = wfc[641:769]

    W1f_x = Wf1[1:]
    b1f = bf1 + bias0 * Wf1[0]
    W1g_x = Wg1[1:]
    b1g = bg1 + bias0 * Wg1[0]

    A2f = Wf2[1:129]
    O2f = Wf2[129:]
    b2f = bf2 + bias0 * Wf2[0]

    wg2 = Wg2 @ v_g2
    wf3 = Wf3 @ v_f3
    wg3 = Wg3 @ v_g3
    G2g = wg2[129:].reshape(NF, NF)
    G3f = wf3[257:].reshape(NF, NF)
    G3g = wg3[257:].reshape(NF, NF)

    ulin_v = W1f_x @ v_ot1 + W1g_x @ v_g1
    vot1_v = wg2[1:129] + wf3[1:129] + wg3[1:129]
    vxf2_v = v_f2 + wf3[129:257] + wg3[129:257]
    call = (bfc[0] + bias0 * (wfc[0] + wg2[0] + wf3[0] + wg3[0])
            + b1f @ v_ot1 + b1g @ v_g1 + bg2 @ v_g2 + bf3 @ v_f3 + bg3 @ v_g3)

    # polarization basis: o1_i o1_j = ((o1_i+o1_j)^2 - o1_i^2 - o1_j^2)/2
    Gt = O2f.reshape(NF, NF, NF)            # [i, j, m]
    A3 = Gt + Gt.transpose(1, 0, 2)         # symmetrized
    iu, ju = np.triu_indices(NF, k=1)       # 8128 pairs i<j
    c_pair = A3[iu, ju, :] / 2.0            # [8128, m]
    rowsum = A3.sum(axis=1)                 # [i, m]
    d_diag = (np.einsum('iim->im', Gt)
              - 0.5 * (rowsum - A3[np.arange(NF), np.arange(NF), :]))

    bf16 = ml_dtypes.bfloat16
    vp_f = np.zeros((NF, NPAD), dtype=bf16)
    pidx = np.arange(len(iu))
    vp_f[iu, pidx] = bf16(1.0)
    vp_f[ju, pidx] = bf16(1.0)
    vp_f[np.arange(NF), 8128 + np.arange(NF)] = bf16(1.0)
    vp = vp_f
    c_full = np.zeros((NPAD, NF))
    c_full[:8128] = c_pair
    c_full[8128:8256] = d_diag
    s2 = np.ascontiguousarray(
        c_full.reshape(NCH, NF, NF).transpose(1, 0, 2).reshape(NF, NPAD)
    ).astype(bf16)

    weights = {
        "w1f": W1f_x.astype(bf16),
        "b1f": b1f.reshape(NF, 1).astype(np.float32),
        "a2f": A2f.astype(bf16),
        "b2f": b2f.reshape(NF, 1).astype(np.float32),
        "vp": vp,
        "s2": s2,
        "g2g": G2g.astype(bf16),
        "g3f": G3f.astype(bf16),
        "g3g": G3g.astype(bf16),
        "ulin": ulin_v.reshape(D_IN, 1).astype(bf16),
        "vot1": vot1_v.reshape(NF, 1).astype(bf16),
        "vxf2": vxf2_v.reshape(NF, 1).astype(bf16),
    }
    return weights, call


def kernel(**inputs):
    import ml_dtypes
    from concourse.bass_utils import run_bass_kernel_spmd

    nc = _get_nc()
    weights, call = _host_fold(inputs)

    x = np.asarray(inputs["x"], np.float32)
    bf16 = ml_dtypes.bfloat16
    in_maps = []
    for c in range(N_CORES):
        shard = np.ascontiguousarray(x[c * R:(c + 1) * R].T).astype(bf16)
        m = dict(weights)
        m["xT"] = shard
        in_maps.append(m)

    res = run_bass_kernel_spmd(nc, in_maps, core_ids=list(range(N_CORES)))
    out = np.empty((B, 1), np.float32)
    for c in range(N_CORES):
        out[c * R:(c + 1) * R, 0] = res.results[c]["out"].reshape(R) + np.float32(call)
    return out
